# revision 1
# baseline (speedup 1.0000x reference)
"""Trainium2 Bass kernel for nn_DiffusionLM (dense_mlp).

Strategy (8 NeuronCores, data-parallel over tokens):
  - 4096 tokens total -> 512 tokens per core; params replicated.
  - Host prep: embedding gather h0 = embed[x], per-step scalar constants,
    step-bias table r1[t] = t_norm[t]*W1[512,:] + b1, and the vocab head
    pre-transposed/gain-folded and cast to bf16: embt = bf16((embed*gn).T).
  - Device per core: 20 reverse-diffusion steps of a 3-layer MLP
    (matmul + LayerNorm + exact gelu) with all activations SBUF-resident,
    token-major layout ([128 tokens, features]); PE transposes feed each
    matmul's stationary operand.  Matmuls run as float32r (full PE rate).
    LayerNorm rstd uses a DVE-side bit-trick rsqrt (keeps the scalar
    engine's activation table pinned to the gelu set).  PSUM tiles are all
    single-bank with a deep shared rotation so the PE never waits on a
    LayerNorm chain.  Then final LayerNorm and the [512,512] @ [512,32000]
    vocab projection in bf16, streaming embt from HBM.
  - Host: concatenate per-core logits -> [2,2048,32000] fp32.
"""

import numpy as np
import ml_dtypes

import concourse.bass as bass
import concourse.mybir as mybir
import concourse.tile as tile
from concourse import bacc, bass_utils
from concourse.bass import ds, ts
from concourse.masks import make_identity

dt = mybir.dt
F32 = dt.float32
F32R = dt.float32r
BF16 = dt.bfloat16
I32 = dt.int32
AF = mybir.ActivationFunctionType
ALU = mybir.AluOpType

# Problem shapes (hardcoded per contract; kernel.py must be self-contained).
N_CORES = 8
VOCAB = 32000
HID = 512
DH = 2 * HID  # 1024
N_STEPS = 20
EPS = 1e-5
B, S = 2, 2048
T_TOTAL = B * S              # 4096
T_CORE = T_TOTAL // N_CORES  # 512
P = 128                      # partitions
RSQRT_MAGIC = 0x5F3759DF


def _step_consts(n_steps):
    """Per-step scalars, ordered t = n_steps-1 .. 0, matching reference."""
    betas = np.linspace(0.0001, 0.02, n_steps, dtype=np.float32)
    alphas = (1.0 - betas).astype(np.float32)
    acp = np.cumprod(alphas, dtype=np.float32)
    tsx = np.arange(n_steps - 1, -1, -1)
    t_norm = (tsx.astype(np.float32) / np.float32(n_steps)).astype(np.float32)
    coef = (betas[tsx] / np.sqrt((np.float32(1.0) - acp[tsx]))).astype(np.float32)
    isa = (np.float32(1.0) / np.sqrt(alphas[tsx])).astype(np.float32)
    return t_norm, coef, isa


def build_program(t_core=T_CORE, n_steps=N_STEPS, vocab=VOCAB,
                  apply_gb1=False, apply_gb2=False,
                  use_b2=False, use_b3=False, use_voff=False):
    """Trace + compile the Bass/Tile program. Returns nc."""
    tp_n = t_core // P  # token tiles per core

    nc = bacc.Bacc("TRN2", target_bir_lowering=False, debug=False,
                   num_devices=N_CORES)

    h0_d = nc.dram_tensor("h0", [t_core, HID], F32, kind="ExternalInput").ap()
    w1_d = nc.dram_tensor("w1", [HID, DH], F32, kind="ExternalInput").ap()
    r1_d = nc.dram_tensor("r1", [1, n_steps, DH], F32,
                          kind="ExternalInput").ap()
    w2_d = nc.dram_tensor("w2", [DH, DH], F32, kind="ExternalInput").ap()
    w3_d = nc.dram_tensor("w3", [DH, HID], F32, kind="ExternalInput").ap()
    emb_d = nc.dram_tensor("embt", [HID, vocab], BF16,
                           kind="ExternalInput").ap()
    out_d = nc.dram_tensor("logits", [t_core, vocab], F32,
                           kind="ExternalOutput").ap()
    b2_d = b3_d = voff_d = None
    if use_b2:
        b2_d = nc.dram_tensor("b2", [1, DH], F32, kind="ExternalInput").ap()
    if use_b3:
        b3_d = nc.dram_tensor("b3", [1, HID], F32, kind="ExternalInput").ap()
    if use_voff:
        voff_d = nc.dram_tensor("voff", [1, vocab], F32,
                                kind="ExternalInput").ap()
    gb_d = None
    if apply_gb1 or apply_gb2:
        gb_d = nc.dram_tensor("gb", [4, DH], F32, kind="ExternalInput").ap()

    t_norm, coef, isa = _step_consts(n_steps)

    def r_(ap):  # f32r view for diffusion matmul operands
        return ap.bitcast(F32R)

    with tile.TileContext(nc) as tc:
      with (
          tc.tile_pool(name="wpool", bufs=1) as wpool,
          tc.tile_pool(name="work", bufs=3) as work,
          tc.tile_pool(name="emb", bufs=4) as embp,
          tc.tile_pool(name="lout", bufs=4) as loutp,
          tc.tile_pool(name="ps", bufs=6, space="PSUM") as psp,
      ):
            # ---- resident constants/weights ----
            ident = wpool.tile([P, P], F32)
            make_identity(nc, ident)
            ones1 = wpool.tile([1, P], F32)
            ones1_f = wpool.tile([1, P], F32, name="ones1_f")
            nc.vector.memset(ones1_f, 1.0)
            nc.vector.tensor_copy(out=r_(ones1), in_=ones1_f)
            magict = wpool.tile([P, 1], I32)
            nc.vector.memset(magict, RSQRT_MAGIC)

            w1s = []
            for kc in range(HID // P):
                w = wpool.tile([P, DH], F32, tag=f"w1_{kc}")
                nc.sync.dma_start(out=r_(w),
                                  in_=r_(w1_d[kc * P:(kc + 1) * P, :]))
                w1s.append(w)
            w2s = []
            for kc in range(DH // P):
                w = wpool.tile([P, DH], F32, tag=f"w2_{kc}")
                nc.sync.dma_start(out=r_(w),
                                  in_=r_(w2_d[kc * P:(kc + 1) * P, :]))
                w2s.append(w)
            w3s = []
            for kc in range(DH // P):
                w = wpool.tile([P, HID], F32, tag=f"w3_{kc}")
                nc.sync.dma_start(out=r_(w),
                                  in_=r_(w3_d[kc * P:(kc + 1) * P, :]))
                w3s.append(w)
            b2s = b3s = voff_s = None
            if use_b2:
                b2s = wpool.tile([1, DH], F32)
                nc.sync.dma_start(out=r_(b2s), in_=r_(b2_d))
            if use_b3:
                b3s = wpool.tile([1, HID], F32)
                nc.sync.dma_start(out=r_(b3s), in_=r_(b3_d))
            if use_voff:
                voff_s = wpool.tile([1, vocab], F32)
                nc.sync.dma_start(out=voff_s, in_=voff_d)
            gbs = None
            if gb_d is not None:
                gbs = wpool.tile([P, 4, DH], F32)
                nc.sync.dma_start(out=gbs, in_=gb_d.to_broadcast([P, 4, DH]))

            hs = []
            for tp in range(tp_n):
                h = wpool.tile([P, HID], F32, tag=f"h_{tp}")
                nc.sync.dma_start(out=h, in_=h0_d[tp * P:(tp + 1) * P, :])
                hs.append(h)
            hcTs = [wpool.tile([P, HID], BF16, tag=f"hcT_{tp}",
                               name=f"hcT_{tp}") for tp in range(tp_n)]

            n_evac = [0]

            def evac(dst, src):
                """PSUM->SBUF copy, rotating between DVE and ACT (2:1)."""
                if n_evac[0] % 3 == 2:
                    nc.scalar.copy(out=dst, in_=src)
                else:
                    nc.vector.tensor_copy(out=dst, in_=src)
                n_evac[0] += 1

            def rsqrt_dve(y, u, tmp, n_iter=2):
                """y = 1/sqrt(u) via bit-trick + Newton, all on DVE.

                y/u/tmp are [P,1] f32 APs."""
                nc.vector.tensor_scalar(out=y.bitcast(I32),
                                        in0=u.bitcast(I32), scalar1=1,
                                        scalar2=None,
                                        op0=ALU.logical_shift_right)
                nc.vector.tensor_tensor(out=y.bitcast(I32), in0=magict,
                                        in1=y.bitcast(I32), op=ALU.subtract)
                for _ in range(n_iter):
                    nc.vector.tensor_tensor(out=tmp, in0=y, in1=y,
                                            op=ALU.mult)
                    nc.vector.tensor_tensor(out=tmp, in0=tmp, in1=u,
                                            op=ALU.mult)
                    nc.vector.tensor_scalar(out=tmp, in0=tmp, scalar1=-0.5,
                                            scalar2=1.5, op0=ALU.mult,
                                            op1=ALU.add)
                    nc.vector.tensor_tensor(out=y, in0=y, in1=tmp,
                                            op=ALU.mult)

            def layernorm_gelu(y, width, gb_idx):
                """In-place y[SBUF f32] <- gelu(LN(y) (*g+be))."""
                nparts = width // 512
                st = work.tile([P, nparts, 6], F32, tag="st")
                for i in range(nparts):
                    nc.vector.bn_stats(out=st[:, i, :],
                                       in_=y[:, ds(i * 512, 512)])
                mv = work.tile([P, 2], F32, tag="mv")
                nc.vector.bn_aggr(out=mv, in_=st)
                # rstd = rsqrt(var + eps) on DVE (no ACT table swap)
                sc2 = work.tile([P, 3], F32, tag="sc2")
                u, rstd, tmp = sc2[:, 0:1], sc2[:, 1:2], sc2[:, 2:3]
                nc.vector.tensor_scalar(out=u, in0=mv[:, 1:2], scalar1=EPS,
                                        scalar2=None, op0=ALU.add)
                rsqrt_dve(rstd, u, tmp)
                for i in range(nparts):
                    nc.vector.tensor_scalar(
                        out=y[:, ds(i * 512, 512)], in0=y[:, ds(i * 512, 512)],
                        scalar1=mv[:, 0:1], scalar2=rstd,
                        op0=ALU.subtract, op1=ALU.mult)
                if gb_idx is not None and gbs is not None:
                    g_t, be_t = gbs[:, gb_idx, :], gbs[:, gb_idx + 1, :]
                    nc.vector.tensor_mul(out=y, in0=y, in1=g_t[:, :width])
                    nc.vector.tensor_add(out=y, in0=y, in1=be_t[:, :width])
                for i in range(nparts):
                    nc.scalar.activation(out=y[:, ds(i * 512, 512)],
                                         in_=y[:, ds(i * 512, 512)],
                                         func=AF.Gelu)

            def transpose_to(src, width, out_dt=F32):
                """SBUF tile [P,width](out_dt) = 128-block transposes of src.

                Uses [P,512] single-bank PSUM tiles."""
                out = work.tile([P, width], out_dt,
                                tag="hT" if width <= 512 else "zT")
                for g in range(width // 512):
                    pt = psp.tile([P, 512], F32, tag="ps")
                    for kc in range(4):
                        c = g * 4 + kc
                        nc.tensor.transpose(pt[:, ts(kc, P)],
                                            src[:, ts(c, P)], ident)
                    dst = out[:, ds(g * 512, 512)]
                    evac(dst if out_dt == BF16 else r_(dst), pt)
                return out

            # ================= diffusion =================
            # Stage-interleaved emission: each engine's program order
            # alternates between the 4 token tiles, so the in-order PE
            # queue always has another tile's matmuls while a LayerNorm
            # chain completes on DVE/ACT.
            def evac_y(y, pps):
                """Stage PSUM halves into SBUF tile y, freeing PSUM early."""
                for i, pp in enumerate(pps):
                    evac(y[:, ds(i * 512, 512)], pp)

            def mlp_layer(xT, ws, n_k, extra):
                """PSUM halves of xT.T @ W (+ optional K=1 extra rows)."""
                pps = []
                for i in range(2):
                    sl = ds(i * 512, 512)
                    pp = psp.tile([P, 512], F32, tag="ps")
                    for kc in range(n_k):
                        nc.tensor.matmul(pp, r_(xT[:, ts(kc, P)]),
                                         r_(ws[kc][:, sl]),
                                         start=(kc == 0),
                                         stop=(kc == n_k - 1 and not extra))
                    for row, tab in (extra or []):
                        nc.tensor.matmul(pp, r_(row), r_(tab[:, sl]),
                                         start=False, stop=True)
                    pps.append(pp)
                return pps

            for step in range(n_steps):
                c_isa = float(coef[step] * isa[step])
                isa_f = float(isa[step])
                r1row = work.tile([1, DH], F32, tag="r1row", bufs=2,
                                  name=f"r1row_{step}")
                nc.sync.dma_start(out=r_(r1row), in_=r_(r1_d[:, step, :]))
                z1s, z2s = {}, {}
                for tp in range(tp_n):
                    hT = transpose_to(hs[tp], HID)
                    pps = mlp_layer(hT, w1s, 4, [(ones1, r1row)])
                    z1 = work.tile([P, DH], F32, tag="zn", name=f"z1_{tp}")
                    evac_y(z1, pps)
                    z1s[tp] = z1
                for tp in range(tp_n):
                    layernorm_gelu(z1s[tp], DH, 0 if apply_gb1 else None)
                    z1T = transpose_to(z1s[tp], DH)
                    extra2 = [(ones1, b2s)] if use_b2 else None
                    pps = mlp_layer(z1T, w2s, 8, extra2)
                    z2 = work.tile([P, DH], F32, tag="zn", name=f"z2_{tp}")
                    evac_y(z2, pps)
                    z2s[tp] = z2
                for tp in range(tp_n):
                    layernorm_gelu(z2s[tp], DH, 2 if apply_gb2 else None)
                    z2T = transpose_to(z2s[tp], DH)
                    ps3 = psp.tile([P, 512], F32, tag="ps")
                    for kc in range(8):
                        last = (kc == 7) and not use_b3
                        nc.tensor.matmul(ps3, r_(z2T[:, ts(kc, P)]),
                                         r_(w3s[kc]),
                                         start=(kc == 0), stop=last)
                    if use_b3:
                        nc.tensor.matmul(ps3, r_(ones1), r_(b3s),
                                         start=False, stop=True)
                    # h = isa*h - (coef*isa)*score
                    h = hs[tp]
                    sc = work.tile([P, HID], F32, tag="sc")
                    nc.vector.tensor_scalar(out=sc, in0=ps3, scalar1=c_isa,
                                            scalar2=None, op0=ALU.mult)
                    hm = work.tile([P, HID], F32, tag="hm")
                    nc.scalar.mul(hm, h, isa_f)
                    nc.gpsimd.tensor_tensor(out=h, in0=hm, in1=sc,
                                            op=ALU.subtract)

            # ============ final LN + hcT (bf16) ============
            for tp in range(tp_n):
                h = hs[tp]
                st = work.tile([P, 6], F32, tag="stf")
                nc.vector.bn_stats(out=st, in_=h)
                mv = work.tile([P, 2], F32, tag="mv")
                nc.vector.bn_aggr(out=mv, in_=st)
                sc2 = work.tile([P, 3], F32, tag="sc2")
                u, rstd, tmp = sc2[:, 0:1], sc2[:, 1:2], sc2[:, 2:3]
                nc.vector.tensor_scalar(out=u, in0=mv[:, 1:2], scalar1=EPS,
                                        scalar2=None, op0=ALU.add)
                rsqrt_dve(rstd, u, tmp)
                hc = work.tile([P, HID], F32, tag="hm")
                nc.vector.tensor_scalar(out=hc, in0=h, scalar1=mv[:, 0:1],
                                        scalar2=rstd, op0=ALU.subtract,
                                        op1=ALU.mult)
                pt = psp.tile([P, 512], F32, tag="ps")
                for kc in range(4):
                    nc.tensor.transpose(pt[:, ts(kc, P)], hc[:, ts(kc, P)],
                                        ident)
                nc.vector.tensor_copy(out=hcTs[tp], in_=pt)

            # ================= logits (bf16) =================
            VC = 2048  # vocab stream chunk (bf16 -> 4KB/partition rows)
            n_vc = (vocab + VC - 1) // VC
            n_out = 0
            for vc in range(n_vc):
                v0 = vc * VC
                vn = min(VC, vocab - v0)
                et = embp.tile([P, 4, vn], BF16, tag="et")
                for kc in range(4):
                    nc.sync.dma_start(
                        out=et[:, kc, :],
                        in_=emb_d[kc * P:(kc + 1) * P, v0:v0 + vn])
                for tp in range(tp_n):
                    for i in range((vn + 511) // 512):
                        w = min(512, vn - i * 512)
                        pl = psp.tile([P, 512], F32, tag="ps")
                        for kc in range(4):
                            last = (kc == 3) and not use_voff
                            nc.tensor.matmul(
                                pl[:, :w], hcTs[tp][:, ts(kc, P)],
                                et[:, kc, ds(i * 512, w)],
                                start=(kc == 0), stop=last)
                        if use_voff:
                            nc.tensor.matmul(
                                pl[:, :w], r_(ones1),
                                r_(voff_s[:, ds(v0 + i * 512, w)]),
                                start=False, stop=True)
                        lo = loutp.tile([P, 512], F32, tag="lo")
                        if n_out % 3 == 0:
                            nc.vector.tensor_copy(out=lo[:, :w], in_=pl[:, :w])
                        else:
                            nc.scalar.copy(out=lo[:, :w], in_=pl[:, :w])
                        n_out += 1
                        nc.sync.dma_start(
                            out=out_d[tp * P:(tp + 1) * P,
                                      v0 + i * 512:v0 + i * 512 + w],
                            in_=lo[:, :w])
    nc.compile()
    return nc


def host_prep(x, embed, W1, b1, g1, be1, W2, b2, g2, be2, W3, b3, gn, bn,
              n_steps=N_STEPS):
    """Pure-numpy input prep shared by all cores."""
    x = np.asarray(x).reshape(-1)
    embed = np.asarray(embed, dtype=np.float32)
    W1 = np.asarray(W1, dtype=np.float32)
    b1 = np.asarray(b1, dtype=np.float32)
    t_norm, _, _ = _step_consts(n_steps)
    h0 = embed[x]                                     # [T_total, HID]
    r1 = (t_norm[:, None] * W1[HID][None, :]
          + b1[None, :]).astype(np.float32)[None]
    gnf = np.asarray(gn, dtype=np.float32)
    embt = np.ascontiguousarray(
        (embed * gnf[None, :]).T.astype(ml_dtypes.bfloat16))  # [HID, VOCAB]
    voff = (np.asarray(bn, dtype=np.float32) @ embed.T).astype(np.float32)
    return dict(
        h0=h0,
        w1=np.ascontiguousarray(W1[:HID]),
        r1=r1,
        w2=np.asarray(W2, dtype=np.float32),
        w3=np.asarray(W3, dtype=np.float32),
        embt=embt,
        b2=np.asarray(b2, dtype=np.float32).reshape(1, -1),
        b3=np.asarray(b3, dtype=np.float32).reshape(1, -1),
        voff=voff.reshape(1, -1),
        g1=np.asarray(g1, dtype=np.float32),
        be1=np.asarray(be1, dtype=np.float32),
        g2=np.asarray(g2, dtype=np.float32),
        be2=np.asarray(be2, dtype=np.float32),
    )


_CACHE = {}


def _get_program(key, **kw):
    if key not in _CACHE:
        _CACHE[key] = build_program(**kw)
    return _CACHE[key]


def kernel(x, embed, W1, b1, g1, be1, W2, b2, g2, be2, W3, b3, gn, bn,
           run_kwargs=None):
    pre = host_prep(x, embed, W1, b1, g1, be1, W2, b2, g2, be2, W3, b3,
                    gn, bn)

    apply_gb1 = bool(np.any(pre["g1"] != 1.0) or np.any(pre["be1"] != 0.0))
    apply_gb2 = bool(np.any(pre["g2"] != 1.0) or np.any(pre["be2"] != 0.0))
    use_b2 = bool(np.any(pre["b2"]))
    use_b3 = bool(np.any(pre["b3"]))
    use_voff = bool(np.any(pre["voff"]))

    key = (apply_gb1, apply_gb2, use_b2, use_b3, use_voff)
    nc = _get_program(key, apply_gb1=apply_gb1, apply_gb2=apply_gb2,
                      use_b2=use_b2, use_b3=use_b3, use_voff=use_voff)

    common = {"w1": pre["w1"], "r1": pre["r1"], "w2": pre["w2"],
              "w3": pre["w3"], "embt": pre["embt"]}
    if use_b2:
        common["b2"] = pre["b2"]
    if use_b3:
        common["b3"] = pre["b3"]
    if use_voff:
        common["voff"] = pre["voff"]
    if apply_gb1 or apply_gb2:
        common["gb"] = np.stack([pre["g1"], pre["be1"], pre["g2"],
                                 pre["be2"]])

    in_maps = []
    for c in range(N_CORES):
        m = dict(common)
        m["h0"] = np.ascontiguousarray(pre["h0"][c * T_CORE:(c + 1) * T_CORE])
        in_maps.append(m)

    res = bass_utils.run_bass_kernel_spmd(
        nc, in_maps, core_ids=list(range(N_CORES)), **(run_kwargs or {}))
    out = np.concatenate([res.results[c]["logits"] for c in range(N_CORES)],
                         axis=0)
    kernel.last_results = res
    return out.reshape(B, S, VOCAB)



# revision 12
# speedup vs baseline: 1.2593x; 1.2593x over previous
"""Trainium2 Bass kernel for nn_DiffusionLM (dense_mlp).

Strategy (8 NeuronCores, data-parallel over tokens; 512 tokens/core):
  - Host: embedding gather + pre-transpose h0 -> h0T [HID, T_CORE] f32;
    weights cast to bf16; step-bias table r1[t] and step coefficient
    folded by the cumulative 1/sqrt(alpha) product A_t (LayerNorm is
    scale-invariant, so the per-step `isa` rescale of h is dropped and
    absorbed into r1/A_t and coef/A_t); vocab head (embed*gn).T in bf16.
  - Device diffusion (20 steps), all bf16 matmuls at 1 cycle/row:
      * h~T kept feature-major in SBUF (f32 master + bf16 copy), so the
        first-layer matmul needs no activation transpose.
      * z1/z2 LayerNorm+gelu fused into a single ACT pass per PSUM half:
        Gelu(psum*rstd + (-mu*rstd)) with per-partition scale/bias APs.
      * z1/z2 transposed back to feature-major by DMA XBAR transposes
        (SBUF->SBUF, off the PE).
      * h~ update is one gpsimd scalar_tensor_tensor: h~ += (-c~)*scoreT.
  - Final LN folded into the vocab projection: center h~T by mu (rank-0
    DMA broadcast of the mu row), fold rstd into the PSUM->SBUF evac.
  - Vocab projection streams embt bf16 from HBM (first chunks prefetched
    during diffusion), writes f32 logits.
"""

import numpy as np
import ml_dtypes

import concourse.bass as bass
import concourse.mybir as mybir
import concourse.tile as tile
from concourse import bacc, bass_utils
from concourse.bass import ds, ts
from concourse.masks import make_identity

dt = mybir.dt
F32 = dt.float32
BF16 = dt.bfloat16
I32 = dt.int32
AF = mybir.ActivationFunctionType
ALU = mybir.AluOpType

N_CORES = 8
VOCAB = 32000
HID = 512
DH = 2 * HID  # 1024
N_STEPS = 20
EPS = 1e-5
B, S = 2, 2048
T_TOTAL = B * S              # 4096
T_CORE = T_TOTAL // N_CORES  # 512
P = 128
TPN = T_CORE // P            # 4 token tiles
KH = HID // P                # 4
KD = DH // P                 # 8
RSQRT_MAGIC = 0x5F3759DF
VC = 2048                    # vocab stream chunk
N_PREF = 3                   # chunks prefetched during diffusion


def _step_consts(n_steps):
    """Per-step scalars, ordered t = n_steps-1 .. 0, matching reference."""
    betas = np.linspace(0.0001, 0.02, n_steps, dtype=np.float32)
    alphas = (1.0 - betas).astype(np.float32)
    acp = np.cumprod(alphas, dtype=np.float32)
    tsx = np.arange(n_steps - 1, -1, -1)
    t_norm = (tsx.astype(np.float32) / np.float32(n_steps)).astype(np.float32)
    coef = (betas[tsx] / np.sqrt((np.float32(1.0) - acp[tsx]))).astype(np.float32)
    isa = (np.float32(1.0) / np.sqrt(alphas[tsx])).astype(np.float32)
    # A_s = prod_{j<s} isa_j; h = A*h~ and LN() erases the final A.
    A = np.ones(n_steps, dtype=np.float64)
    for s_ in range(1, n_steps):
        A[s_] = A[s_ - 1] * isa[s_ - 1]
    ctil = (coef.astype(np.float64) / A).astype(np.float32)
    return t_norm, coef, isa, A.astype(np.float32), ctil


def build_program(n_steps=N_STEPS, vocab=VOCAB,
                  apply_gb1=False, apply_gb2=False,
                  use_b2=False, use_b3=False, use_voff=False):
    nc = bacc.Bacc("TRN2", target_bir_lowering=False, debug=False,
                   num_devices=N_CORES)

    h0t_d = nc.dram_tensor("h0t", [HID, T_CORE], F32, kind="ExternalInput").ap()
    w1_d = nc.dram_tensor("w1", [HID, DH], BF16, kind="ExternalInput").ap()
    r1_d = nc.dram_tensor("r1", [1, n_steps, DH], BF16,
                          kind="ExternalInput").ap()
    w2_d = nc.dram_tensor("w2", [DH, DH], BF16, kind="ExternalInput").ap()
    w3_d = nc.dram_tensor("w3", [DH, HID], BF16, kind="ExternalInput").ap()
    emb_d = nc.dram_tensor("embt", [HID, vocab], BF16,
                           kind="ExternalInput").ap()
    out_d = nc.dram_tensor("logits", [T_CORE, vocab], F32,
                           kind="ExternalOutput").ap()
    b2_d = b3_d = voff_d = gb_d = None
    if use_b2:
        b2_d = nc.dram_tensor("b2", [1, DH], BF16, kind="ExternalInput").ap()
    if use_b3:
        b3_d = nc.dram_tensor("b3", [1, HID], BF16, kind="ExternalInput").ap()
    if use_voff:
        voff_d = nc.dram_tensor("voff", [1, vocab], F32,
                                kind="ExternalInput").ap()
    if apply_gb1 or apply_gb2:
        gb_d = nc.dram_tensor("gb", [4, DH], F32, kind="ExternalInput").ap()
    mu_d = nc.dram_tensor("mu_scratch", [1, T_CORE], F32,
                          kind="Internal").ap()

    _, _, _, _, ctil = _step_consts(n_steps)

    with tile.TileContext(nc) as tc:
      with (
          tc.tile_pool(name="wpool", bufs=1) as wpool,
          tc.tile_pool(name="work", bufs=3) as work,
          tc.tile_pool(name="emb", bufs=N_PREF) as embp,
          tc.tile_pool(name="lout", bufs=4) as loutp,
          tc.tile_pool(name="ps", bufs=8, space="PSUM") as psp,
      ):
            # ---- resident constants / weights ----
            ones1 = wpool.tile([1, P], BF16)
            nc.vector.memset(ones1, 1.0)
            ident = wpool.tile([P, P], F32)
            make_identity(nc, ident)

            w1s = []
            for kc in range(KH):
                w = wpool.tile([P, DH], BF16, tag=f"w1_{kc}")
                nc.sync.dma_start(out=w, in_=w1_d[kc * P:(kc + 1) * P, :])
                w1s.append(w)
            w2s = []
            for kc in range(KD):
                w = wpool.tile([P, DH], BF16, tag=f"w2_{kc}")
                nc.sync.dma_start(out=w, in_=w2_d[kc * P:(kc + 1) * P, :])
                w2s.append(w)
            w3s = []
            for kc in range(KD):
                w = wpool.tile([P, HID], BF16, tag=f"w3_{kc}")
                nc.sync.dma_start(out=w, in_=w3_d[kc * P:(kc + 1) * P, :])
                w3s.append(w)

            b2s = b3s = voff_s = onesrow = gbs = None
            if use_b2:
                b2s = wpool.tile([1, DH], BF16)
                nc.sync.dma_start(out=b2s, in_=b2_d)
            if use_b3:
                b3s = wpool.tile([1, HID], BF16)
                nc.sync.dma_start(out=b3s, in_=b3_d)
                onesrow = wpool.tile([1, T_CORE], BF16)
                nc.vector.memset(onesrow, 1.0)
            if use_voff:
                voff_s = wpool.tile([1, vocab], F32)
                nc.sync.dma_start(out=voff_s, in_=voff_d)
                voff_bc = wpool.tile([P, VC], F32)
            if gb_d is not None:
                gbs = wpool.tile([P, 4, DH], F32)
                nc.sync.dma_start(out=gbs, in_=gb_d.to_broadcast([P, 4, DH]))

            # persistent h~T (feature-major): f32 master + bf16 matmul copy
            hT = wpool.tile([P, KH, T_CORE], F32)
            for kc in range(KH):
                nc.sync.dma_start(out=hT[:, kc, :],
                                  in_=h0t_d[kc * P:(kc + 1) * P, :])
            hbf = wpool.tile([P, KH, T_CORE], BF16)
            for kc in range(KH):
                nc.scalar.copy(out=hbf[:, kc, :], in_=hT[:, kc, :])
            hcT = wpool.tile([P, KH, T_CORE], BF16)
            magict = wpool.tile([P, TPN], I32)
            nc.vector.memset(magict, RSQRT_MAGIC)

            def rsqrt_chain(mvp, n):
                """DVE chain on [P,n]: returns (rstd, negbias) tiles.

                mvp is [P,n,2] f32 (mean, var) from bn_aggr."""
                u = work.tile([P, n], F32, tag="u", bufs=4)
                yv = work.tile([P, n], F32, tag="yv", bufs=4)
                t2 = work.tile([P, n], F32, tag="t2", bufs=4)
                nb = work.tile([P, n], F32, tag="nb", bufs=4)
                nc.vector.tensor_scalar(out=u, in0=mvp[:, :, 1], scalar1=EPS,
                                        scalar2=None, op0=ALU.add)
                nc.vector.tensor_scalar(out=t2.bitcast(I32),
                                        in0=u.bitcast(I32), scalar1=1,
                                        scalar2=None,
                                        op0=ALU.logical_shift_right)
                nc.vector.tensor_tensor(out=yv.bitcast(I32),
                                        in0=magict[:, :n],
                                        in1=t2.bitcast(I32), op=ALU.subtract)
                # Newton 1: rstd = est * (1.5 - 0.5*u*est^2)
                nc.vector.tensor_tensor(out=t2, in0=yv, in1=yv, op=ALU.mult)
                nc.vector.tensor_tensor(out=t2, in0=t2, in1=u, op=ALU.mult)
                nc.vector.tensor_scalar(out=t2, in0=t2, scalar1=-0.5,
                                        scalar2=1.5, op0=ALU.mult, op1=ALU.add)
                nc.vector.tensor_tensor(out=yv, in0=yv, in1=t2, op=ALU.mult)
                # nb = -mean * rstd
                nc.vector.scalar_tensor_tensor(out=nb, in0=mvp[:, :, 0],
                                               scalar=-1.0, in1=yv,
                                               op0=ALU.mult, op1=ALU.mult)
                return yv, nb

            n_eng = [0]

            def ln_gelu_pair(pps, sts, tpair, zts, gb_idx):
                """Stats-chain + fused LN/gelu for two token tiles."""
                mvp = work.tile([P, 2, 2], F32, tag="mv", bufs=4)
                for i, t in enumerate(tpair):
                    nc.vector.bn_aggr(out=mvp[:, i, :], in_=sts[t])
                rstd, nb = rsqrt_chain(mvp, 2)
                for i, t in enumerate(tpair):
                    if gb_idx is None:
                        for h in range(2):
                            nc.scalar.activation(
                                out=zts[t][:, ds(h * 512, 512)],
                                in_=pps[t][h], func=AF.Gelu,
                                scale=rstd[:, i:i + 1], bias=nb[:, i:i + 1])
                    else:
                        # general path: g/be per-feature after LN
                        zf = work.tile([P, DH], F32, tag="zf", bufs=2)
                        for h in range(2):
                            nc.vector.tensor_scalar(
                                out=zf[:, ds(h * 512, 512)], in0=pps[t][h],
                                scalar1=mvp[:, i, 0:1],
                                scalar2=rstd[:, i:i + 1],
                                op0=ALU.subtract, op1=ALU.mult)
                        g_t = gbs[:, gb_idx, :]
                        be_t = gbs[:, gb_idx + 1, :]
                        nc.vector.tensor_tensor(out=zf, in0=zf, in1=g_t,
                                                op=ALU.mult)
                        nc.vector.tensor_tensor(out=zf, in0=zf, in1=be_t,
                                                op=ALU.add)
                        nc.scalar.activation(out=zts[t], in_=zf, func=AF.Gelu)

            # ================= diffusion =================
            ets = []
            for step in range(n_steps):
                cneg = -float(ctil[step])
                r1row = work.tile([1, DH], BF16, tag="r1row", bufs=2,
                                  name=f"r1row_{step}")
                nc.sync.dma_start(out=r1row, in_=r1_d[:, step, :])

                # ---- layer 1: z1 = gelu(LN(h~ @ W1 + r1~)) ----
                z1ps, z1st, z1t = {}, {}, {}
                z1T = work.tile([P, KD, T_CORE], BF16, tag="z1T", bufs=2,
                                name=f"z1T_{step}")
                for tp in range(TPN):
                    pp = [psp.tile([P, 512], F32, tag="ps",
                                   name=f"ps1_{step}_{tp}_{h}")
                          for h in range(2)]
                    for kc in range(KH):
                        for h in range(2):
                            nc.tensor.matmul(pp[h], hbf[:, kc, ts(tp, P)],
                                             w1s[kc][:, ds(h * 512, 512)],
                                             start=(kc == 0), stop=False)
                    for h in range(2):
                        nc.tensor.matmul(pp[h], ones1,
                                         r1row[:, ds(h * 512, 512)],
                                         start=False, stop=True)
                    st = work.tile([P, 2, 6], F32, tag="st", bufs=4)
                    for h in range(2):
                        nc.vector.bn_stats(out=st[:, h, :], in_=pp[h])
                    z1ps[tp], z1st[tp] = pp, st
                    z1t[tp] = work.tile([P, DH], BF16, tag="z", bufs=3,
                                        name=f"z1_{step}_{tp}")
                    if tp % 2 == 1:
                        pair = (tp - 1, tp)
                        ln_gelu_pair(z1ps, z1st, pair, z1t,
                                     0 if apply_gb1 else None)
                        for t in pair:
                            nc.sync.dma_start(out=z1T[:, :, ts(t, P)],
                                              in_=z1t[t], transpose=True)

                # ---- layer 2: z2 = gelu(LN(z1 @ W2 (+b2))) ----
                z2ps, z2st, z2t = {}, {}, {}
                z2T = work.tile([P, KD, T_CORE], BF16, tag="z2T", bufs=2,
                                name=f"z2T_{step}")
                for tp in range(TPN):
                    pp = [psp.tile([P, 512], F32, tag="ps",
                                   name=f"ps2_{step}_{tp}_{h}")
                          for h in range(2)]
                    for kc in range(KD):
                        for h in range(2):
                            nc.tensor.matmul(pp[h], z1T[:, kc, ts(tp, P)],
                                             w2s[kc][:, ds(h * 512, 512)],
                                             start=(kc == 0),
                                             stop=(kc == KD - 1 and not use_b2))
                    if use_b2:
                        for h in range(2):
                            nc.tensor.matmul(pp[h], ones1,
                                             b2s[:, ds(h * 512, 512)],
                                             start=False, stop=True)
                    st = work.tile([P, 2, 6], F32, tag="st", bufs=4)
                    for h in range(2):
                        nc.vector.bn_stats(out=st[:, h, :], in_=pp[h])
                    z2ps[tp], z2st[tp] = pp, st
                    z2t[tp] = work.tile([P, DH], BF16, tag="z", bufs=3,
                                        name=f"z2_{step}_{tp}")
                    if tp % 2 == 1:
                        pair = (tp - 1, tp)
                        ln_gelu_pair(z2ps, z2st, pair, z2t,
                                     2 if apply_gb2 else None)
                        for t in pair:
                            nc.sync.dma_start(out=z2T[:, :, ts(t, P)],
                                              in_=z2t[t], transpose=True)

                # ---- layer 3 (feature-major) + h~ update ----
                ps3 = [psp.tile([P, 512], F32, tag="ps",
                                name=f"ps3_{step}_{mc}")
                       for mc in range(KH)]
                for hn in range(2):  # token halves so PE starts earlier
                    sl = ds(hn * 256, 256)
                    for mc in range(KH):
                        for kc in range(KD):
                            nc.tensor.matmul(
                                ps3[mc][:, sl], w3s[kc][:, ts(mc, P)],
                                z2T[:, kc, sl],
                                start=(kc == 0),
                                stop=(kc == KD - 1 and not use_b3))
                        if use_b3:
                            nc.tensor.matmul(ps3[mc][:, sl],
                                             b3s[:, ts(mc, P)], onesrow[:, sl],
                                             start=False, stop=True)
                for mc in range(KH):
                    nc.vector.scalar_tensor_tensor(
                        out=hT[:, mc, :], in0=ps3[mc], scalar=cneg,
                        in1=hT[:, mc, :], op0=ALU.mult, op1=ALU.add)
                    if step < n_steps - 1:
                        nc.scalar.copy(out=hbf[:, mc, :], in_=hT[:, mc, :])

                # prefetch first embt chunks late in diffusion
                if n_steps - 1 - N_PREF <= step < n_steps - 1:
                    vc = step - (n_steps - 1 - N_PREF)
                    et = embp.tile([P, KH, VC], BF16, tag="et")
                    for kc in range(KH):
                        nc.sync.dma_start(
                            out=et[:, kc, :],
                            in_=emb_d[kc * P:(kc + 1) * P,
                                      vc * VC:(vc + 1) * VC])
                    ets.append(et)

            # ============ final LN (folded into vocab head) ============
            # token-major copy of h~ for per-token stats
            htok = wpool.tile([P, TPN, KH, P], BF16)
            for kc in range(KH):
                nc.scalar.copy(out=hbf[:, kc, :], in_=hT[:, kc, :])
                nc.sync.dma_start(out=htok[:, :, kc, :], in_=hbf[:, kc, :],
                                  transpose=True)
            mvf = wpool.tile([P, TPN, 2], F32)
            for tp in range(TPN):
                stf = work.tile([P, KH, 6], F32, tag="stf", bufs=4)
                for kc in range(KH):
                    nc.vector.bn_stats(out=stf[:, kc, :],
                                       in_=htok[:, tp, kc, :])
                nc.vector.bn_aggr(out=mvf[:, tp, :], in_=stf)
            rsf, _nbf = rsqrt_chain(mvf, TPN)
            # mu row -> [1, T_CORE] via PE transpose + DMA, then broadcast
            mu4 = wpool.tile([P, TPN], F32)
            nc.vector.tensor_copy(out=mu4, in_=mvf[:, :, 0])
            ptm = psp.tile([P, 512], F32, tag="ps")
            nc.tensor.transpose(ptm[0:TPN, 0:P], mu4, ident)
            mur4 = wpool.tile([P, P], F32, name="mur4")
            nc.vector.tensor_copy(out=mur4[0:TPN, :], in_=ptm[0:TPN, 0:P])
            nc.sync.dma_start(out=mu_d, in_=mur4[0:TPN, :])
            mu_bc = wpool.tile([P, T_CORE], F32)
            nc.sync.dma_start(out=mu_bc, in_=mu_d.to_broadcast([P, T_CORE]))
            for kc in range(KH):
                nc.vector.tensor_tensor(out=hcT[:, kc, :], in0=hT[:, kc, :],
                                        in1=mu_bc, op=ALU.subtract)

            # ================= vocab head =================
            n_vc = (vocab + VC - 1) // VC
            n_out = [0]

            def evac_logits(lo_sl, pl_sl, tp):
                rst = rsf[:, tp:tp + 1]
                if n_out[0] % 2 == 0:
                    nc.vector.tensor_scalar(out=lo_sl, in0=pl_sl, scalar1=rst,
                                            scalar2=None, op0=ALU.mult)
                else:
                    nc.scalar.mul(lo_sl, pl_sl, rst)
                n_out[0] += 1

            for vc in range(n_vc):
                v0 = vc * VC
                vn = min(VC, vocab - v0)
                if vc < len(ets):
                    et = ets[vc]
                else:
                    et = embp.tile([P, KH, VC], BF16, tag="et")
                    for kc in range(KH):
                        nc.sync.dma_start(
                            out=et[:, kc, :vn],
                            in_=emb_d[kc * P:(kc + 1) * P, v0:v0 + vn])
                if use_voff:
                    nc.sync.dma_start(
                        out=voff_bc[:, :vn],
                        in_=voff_s[:, v0:v0 + vn].to_broadcast([P, vn]))
                nsl = (vn + 511) // 512
                for tp in range(TPN):
                    # pairs of 512-slices share one lout tile + one DMA out
                    for i0 in range(0, nsl, 2):
                        sls = [i for i in (i0, i0 + 1) if i < nsl]
                        ws = [min(512, vn - i * 512) for i in sls]
                        wtot = sum(ws)
                        pls = [psp.tile([P, 512], F32, tag="ps",
                                        name=f"plv_{vc}_{tp}_{i}")
                               for i in sls]
                        for kc in range(KH):
                            for j, i in enumerate(sls):
                                nc.tensor.matmul(
                                    pls[j][:, :ws[j]], hcT[:, kc, ts(tp, P)],
                                    et[:, kc, ds(i * 512, ws[j])],
                                    start=(kc == 0), stop=(kc == KH - 1))
                        lo = loutp.tile([P, 1024], F32, tag="lo")
                        off = 0
                        for j in range(len(sls)):
                            evac_logits(lo[:, ds(off, ws[j])],
                                        pls[j][:, :ws[j]], tp)
                            off += ws[j]
                        if use_voff:
                            nc.vector.tensor_tensor(
                                out=lo[:, :wtot], in0=lo[:, :wtot],
                                in1=voff_bc[:, ds(i0 * 512, wtot)],
                                op=ALU.add)
                        nc.sync.dma_start(
                            out=out_d[tp * P:(tp + 1) * P,
                                      v0 + i0 * 512:v0 + i0 * 512 + wtot],
                            in_=lo[:, :wtot])
    nc.compile()
    return nc


def host_prep(x, embed, W1, b1, g1, be1, W2, b2, g2, be2, W3, b3, gn, bn,
              n_steps=N_STEPS):
    """Pure-numpy input prep shared by all cores."""
    x = np.asarray(x).reshape(-1)
    embed = np.asarray(embed, dtype=np.float32)
    W1 = np.asarray(W1, dtype=np.float32)
    b1 = np.asarray(b1, dtype=np.float32)
    t_norm, _, _, A, _ = _step_consts(n_steps)
    h0 = embed[x]                                     # [T_total, HID]
    r1 = ((t_norm[:, None] * W1[HID][None, :] + b1[None, :])
          / A[:, None]).astype(ml_dtypes.bfloat16)[None]
    gnf = np.asarray(gn, dtype=np.float32)
    embt = np.ascontiguousarray(
        (embed * gnf[None, :]).T.astype(ml_dtypes.bfloat16))  # [HID, VOCAB]
    voff = (np.asarray(bn, dtype=np.float32) @ embed.T).astype(np.float32)
    return dict(
        h0=np.ascontiguousarray(h0),
        w1=np.ascontiguousarray(W1[:HID]).astype(ml_dtypes.bfloat16),
        r1=np.ascontiguousarray(r1),
        w2=np.asarray(W2, dtype=np.float32).astype(ml_dtypes.bfloat16),
        w3=np.asarray(W3, dtype=np.float32).astype(ml_dtypes.bfloat16),
        embt=embt,
        b2=np.asarray(b2, dtype=np.float32).astype(
            ml_dtypes.bfloat16).reshape(1, -1),
        b3=np.asarray(b3, dtype=np.float32).astype(
            ml_dtypes.bfloat16).reshape(1, -1),
        voff=voff.reshape(1, -1),
        g1=np.asarray(g1, dtype=np.float32),
        be1=np.asarray(be1, dtype=np.float32),
        g2=np.asarray(g2, dtype=np.float32),
        be2=np.asarray(be2, dtype=np.float32),
    )


_CACHE = {}


def _get_program(key, **kw):
    if key not in _CACHE:
        _CACHE[key] = build_program(**kw)
    return _CACHE[key]


def kernel(x, embed, W1, b1, g1, be1, W2, b2, g2, be2, W3, b3, gn, bn,
           run_kwargs=None):
    pre = host_prep(x, embed, W1, b1, g1, be1, W2, b2, g2, be2, W3, b3,
                    gn, bn)

    apply_gb1 = bool(np.any(pre["g1"] != 1.0) or np.any(pre["be1"] != 0.0))
    apply_gb2 = bool(np.any(pre["g2"] != 1.0) or np.any(pre["be2"] != 0.0))
    use_b2 = bool(np.any(np.asarray(b2)))
    use_b3 = bool(np.any(np.asarray(b3)))
    use_voff = bool(np.any(pre["voff"]))

    key = (apply_gb1, apply_gb2, use_b2, use_b3, use_voff)
    nc = _get_program(key, apply_gb1=apply_gb1, apply_gb2=apply_gb2,
                      use_b2=use_b2, use_b3=use_b3, use_voff=use_voff)

    common = {"w1": pre["w1"], "r1": pre["r1"], "w2": pre["w2"],
              "w3": pre["w3"], "embt": pre["embt"]}
    if use_b2:
        common["b2"] = pre["b2"]
    if use_b3:
        common["b3"] = pre["b3"]
    if use_voff:
        common["voff"] = pre["voff"]
    if apply_gb1 or apply_gb2:
        common["gb"] = np.stack([pre["g1"], pre["be1"], pre["g2"],
                                 pre["be2"]])

    in_maps = []
    for c in range(N_CORES):
        m = dict(common)
        m["h0t"] = np.ascontiguousarray(
            pre["h0"][c * T_CORE:(c + 1) * T_CORE].T)
        in_maps.append(m)

    res = bass_utils.run_bass_kernel_spmd(
        nc, in_maps, core_ids=list(range(N_CORES)), **(run_kwargs or {}))
    out = np.concatenate([res.results[c]["logits"] for c in range(N_CORES)],
                         axis=0)
    kernel.last_results = res
    return out.reshape(B, S, VOCAB)


# revision 21
# speedup vs baseline: 1.3112x; 1.0413x over previous
"""Trainium2 Bass kernel for nn_DiffusionLM (dense_mlp).

Strategy (8 NeuronCores, data-parallel over tokens; 512 tokens/core):
  - Host: embedding gather + pre-transpose h0 -> h0T [HID, T_CORE] f32;
    weights cast to bf16; step-bias table r1[t] and step coefficient
    folded by the cumulative 1/sqrt(alpha) product A_t (LayerNorm is
    scale-invariant, so the per-step `isa` rescale of h is dropped and
    absorbed into r1/A_t and coef/A_t); vocab head (embed*gn).T in bf16.
  - Device diffusion (20 steps), all bf16 matmuls at 1 cycle/row:
      * h~T kept feature-major in SBUF (f32 master + bf16 copy), so the
        first-layer matmul needs no activation transpose.
      * z1/z2 LayerNorm+gelu fused into a single ACT pass per PSUM half:
        Gelu(psum*rstd + (-mu*rstd)) with per-partition scale/bias APs.
      * z1/z2 transposed back to feature-major by DMA XBAR transposes
        (SBUF->SBUF, off the PE).
      * h~ update is one gpsimd scalar_tensor_tensor: h~ += (-c~)*scoreT.
  - Final LN folded into the vocab projection: center h~T by mu (rank-0
    DMA broadcast of the mu row), fold rstd into the PSUM->SBUF evac.
  - Vocab projection streams embt bf16 from HBM (first chunks prefetched
    during diffusion), writes f32 logits.
"""

import numpy as np
import ml_dtypes

import concourse.bass as bass
import concourse.mybir as mybir
import concourse.tile as tile
from concourse import bacc, bass_utils
from concourse.bass import ds, ts
from concourse.masks import make_identity

dt = mybir.dt
F32 = dt.float32
BF16 = dt.bfloat16
I32 = dt.int32
AF = mybir.ActivationFunctionType
ALU = mybir.AluOpType

N_CORES = 8
VOCAB = 32000
HID = 512
DH = 2 * HID  # 1024
N_STEPS = 20
EPS = 1e-5
B, S = 2, 2048
T_TOTAL = B * S              # 4096
T_CORE = T_TOTAL // N_CORES  # 512
P = 128
TPN = T_CORE // P            # 4 token tiles
KH = HID // P                # 4
KD = DH // P                 # 8
RSQRT_MAGIC = 0x5F3759DF
VC = 2048                    # vocab stream chunk
N_PREF = 3                   # chunks prefetched during diffusion
EMB_BUFS = 4                 # embt chunk ring size


def _step_consts(n_steps):
    """Per-step scalars, ordered t = n_steps-1 .. 0, matching reference."""
    betas = np.linspace(0.0001, 0.02, n_steps, dtype=np.float32)
    alphas = (1.0 - betas).astype(np.float32)
    acp = np.cumprod(alphas, dtype=np.float32)
    tsx = np.arange(n_steps - 1, -1, -1)
    t_norm = (tsx.astype(np.float32) / np.float32(n_steps)).astype(np.float32)
    coef = (betas[tsx] / np.sqrt((np.float32(1.0) - acp[tsx]))).astype(np.float32)
    isa = (np.float32(1.0) / np.sqrt(alphas[tsx])).astype(np.float32)
    # A_s = prod_{j<s} isa_j; h = A*h~ and LN() erases the final A.
    A = np.ones(n_steps, dtype=np.float64)
    for s_ in range(1, n_steps):
        A[s_] = A[s_ - 1] * isa[s_ - 1]
    ctil = (coef.astype(np.float64) / A).astype(np.float32)
    return t_norm, coef, isa, A.astype(np.float32), ctil


def build_program(n_steps=N_STEPS, vocab=VOCAB,
                  apply_gb1=False, apply_gb2=False,
                  use_b2=False, use_b3=False, use_voff=False):
    nc = bacc.Bacc("TRN2", target_bir_lowering=False, debug=False,
                   num_devices=N_CORES)

    h0t_d = nc.dram_tensor("h0t", [HID, T_CORE], F32, kind="ExternalInput").ap()
    w1_d = nc.dram_tensor("w1", [HID, DH], BF16, kind="ExternalInput").ap()
    r1_d = nc.dram_tensor("r1", [1, n_steps, DH], BF16,
                          kind="ExternalInput").ap()
    w2_d = nc.dram_tensor("w2", [DH, DH], BF16, kind="ExternalInput").ap()
    w3_d = nc.dram_tensor("w3", [DH, HID], BF16, kind="ExternalInput").ap()
    emb_d = nc.dram_tensor("embt", [HID, vocab], BF16,
                           kind="ExternalInput").ap()
    out_d = nc.dram_tensor("logits", [T_CORE, vocab], F32,
                           kind="ExternalOutput").ap()
    b2_d = b3_d = voff_d = gb_d = None
    if use_b2:
        b2_d = nc.dram_tensor("b2", [1, DH], BF16, kind="ExternalInput").ap()
    if use_b3:
        b3_d = nc.dram_tensor("b3", [1, HID], BF16, kind="ExternalInput").ap()
    if use_voff:
        voff_d = nc.dram_tensor("voff", [1, vocab], F32,
                                kind="ExternalInput").ap()
    if apply_gb1 or apply_gb2:
        gb_d = nc.dram_tensor("gb", [4, DH], F32, kind="ExternalInput").ap()
    mu_d = nc.dram_tensor("mu_scratch", [1, T_CORE], F32,
                          kind="Internal").ap()

    _, _, _, _, ctil = _step_consts(n_steps)

    with tile.TileContext(nc) as tc:
      with (
          tc.tile_pool(name="wpool", bufs=1) as wpool,
          tc.tile_pool(name="work", bufs=3) as work,
          tc.tile_pool(name="emb", bufs=EMB_BUFS) as embp,
          tc.tile_pool(name="lout", bufs=4) as loutp,
          tc.tile_pool(name="ps", bufs=8, space="PSUM") as psp,
      ):
            # ---- resident constants / weights ----
            ones1 = wpool.tile([1, P], BF16)
            nc.vector.memset(ones1, 1.0)
            ident = wpool.tile([P, P], F32)
            make_identity(nc, ident)

            # h0/w1 first on SP queue (needed by step 0); w2/w3 on ACT queue
            hT = wpool.tile([P, KH, T_CORE], F32)
            for kc in range(KH):
                nc.sync.dma_start(out=hT[:, kc, :],
                                  in_=h0t_d[kc * P:(kc + 1) * P, :])
            w1s = []
            for kc in range(KH):
                w = wpool.tile([P, DH], BF16, tag=f"w1_{kc}")
                nc.sync.dma_start(out=w, in_=w1_d[kc * P:(kc + 1) * P, :])
                w1s.append(w)
            w2s = []
            for kc in range(KD):
                w = wpool.tile([P, DH], BF16, tag=f"w2_{kc}")
                nc.scalar.dma_start(out=w, in_=w2_d[kc * P:(kc + 1) * P, :])
                w2s.append(w)
            w3s = []
            for kc in range(KD):
                w = wpool.tile([P, HID], BF16, tag=f"w3_{kc}")
                nc.scalar.dma_start(out=w, in_=w3_d[kc * P:(kc + 1) * P, :])
                w3s.append(w)

            b2s = b3s = voff_s = onesrow = gbs = None
            if use_b2:
                b2s = wpool.tile([1, DH], BF16)
                nc.sync.dma_start(out=b2s, in_=b2_d)
            if use_b3:
                b3s = wpool.tile([1, HID], BF16)
                nc.sync.dma_start(out=b3s, in_=b3_d)
                onesrow = wpool.tile([1, T_CORE], BF16)
                nc.vector.memset(onesrow, 1.0)
            if use_voff:
                voff_s = wpool.tile([1, vocab], F32)
                nc.sync.dma_start(out=voff_s, in_=voff_d)
                voff_bc = wpool.tile([P, VC], F32)
            if gb_d is not None:
                gbs = wpool.tile([P, 4, DH], F32)
                nc.sync.dma_start(out=gbs, in_=gb_d.to_broadcast([P, 4, DH]))

            # persistent h~T (feature-major): f32 master + bf16 matmul copy
            hbf = wpool.tile([P, KH, T_CORE], BF16)
            for kc in range(KH):
                nc.scalar.copy(out=hbf[:, kc, :], in_=hT[:, kc, :])
            hcT = wpool.tile([P, KH, T_CORE], BF16)
            magict = wpool.tile([P, TPN], I32)
            nc.vector.memset(magict, RSQRT_MAGIC)

            def rsqrt_chain(mvp, n):
                """DVE chain on [P,n]: returns (rstd, negbias) tiles.

                mvp is [P,n,2] f32 (mean, var) from bn_aggr."""
                u = work.tile([P, n], F32, tag="u", bufs=4)
                yv = work.tile([P, n], F32, tag="yv", bufs=4)
                t2 = work.tile([P, n], F32, tag="t2", bufs=4)
                nb = work.tile([P, n], F32, tag="nb", bufs=4)
                nc.vector.tensor_scalar(out=u, in0=mvp[:, :, 1], scalar1=EPS,
                                        scalar2=None, op0=ALU.add)
                nc.vector.tensor_scalar(out=t2.bitcast(I32),
                                        in0=u.bitcast(I32), scalar1=1,
                                        scalar2=None,
                                        op0=ALU.logical_shift_right)
                nc.vector.tensor_tensor(out=yv.bitcast(I32),
                                        in0=magict[:, :n],
                                        in1=t2.bitcast(I32), op=ALU.subtract)
                # Newton 1: rstd = est * (1.5 - 0.5*u*est^2)
                nc.vector.tensor_tensor(out=t2, in0=yv, in1=yv, op=ALU.mult)
                nc.vector.tensor_tensor(out=t2, in0=t2, in1=u, op=ALU.mult)
                nc.vector.tensor_scalar(out=t2, in0=t2, scalar1=-0.5,
                                        scalar2=1.5, op0=ALU.mult, op1=ALU.add)
                nc.vector.tensor_tensor(out=yv, in0=yv, in1=t2, op=ALU.mult)
                # nb = -mean * rstd
                nc.vector.scalar_tensor_tensor(out=nb, in0=mvp[:, :, 0],
                                               scalar=-1.0, in1=yv,
                                               op0=ALU.mult, op1=ALU.mult)
                return yv, nb

            # LN-chain emission groups: tile 0 alone (unblocks ACT/PE
            # early), then (1,2), then 3.
            LN_GROUPS = {0: (0,), 2: (1, 2), 3: (3,)}

            def ln_gelu_group(pps, sts, group, zts, zT, gb_idx):
                """Stats-chain + fused LN/gelu + DMA transpose per group."""
                ng = len(group)
                mvp = work.tile([P, ng, 2], F32, tag=f"mv{ng}", bufs=4)
                for i, t in enumerate(group):
                    nc.vector.bn_aggr(out=mvp[:, i, :], in_=sts[t])
                rstd, nb = rsqrt_chain(mvp, ng)
                for i, t in enumerate(group):
                    if gb_idx is None:
                        for h in range(2):
                            nc.scalar.activation(
                                out=zts[t][:, ds(h * 512, 512)],
                                in_=pps[t][h], func=AF.Gelu,
                                scale=rstd[:, i:i + 1], bias=nb[:, i:i + 1])
                            nc.sync.dma_start(
                                out=zT[:, ds(h * KH, KH), ts(t, P)],
                                in_=zts[t][:, ds(h * 512, 512)],
                                transpose=True)
                    else:
                        # general path: g/be per-feature after LN
                        zf = work.tile([P, DH], F32, tag="zf", bufs=2)
                        for h in range(2):
                            nc.vector.tensor_scalar(
                                out=zf[:, ds(h * 512, 512)], in0=pps[t][h],
                                scalar1=mvp[:, i, 0:1],
                                scalar2=rstd[:, i:i + 1],
                                op0=ALU.subtract, op1=ALU.mult)
                        g_t = gbs[:, gb_idx, :]
                        be_t = gbs[:, gb_idx + 1, :]
                        nc.vector.tensor_tensor(out=zf, in0=zf, in1=g_t,
                                                op=ALU.mult)
                        nc.vector.tensor_tensor(out=zf, in0=zf, in1=be_t,
                                                op=ALU.add)
                        nc.scalar.activation(out=zts[t], in_=zf, func=AF.Gelu)
                        nc.sync.dma_start(out=zT[:, :, ts(t, P)], in_=zts[t],
                                          transpose=True)

            # ================= diffusion =================
            ets = []

            def load_et(vc):
                v0e = vc * VC
                vne = min(VC, vocab - v0e)
                et = embp.tile([P, KH, VC], BF16, tag="et",
                               name=f"et_{vc}")
                for kc in range(KH):
                    nc.sync.dma_start(
                        out=et[:, kc, :vne],
                        in_=emb_d[kc * P:(kc + 1) * P, v0e:v0e + vne])
                ets.append(et)
            for step in range(n_steps):
                cneg = -float(ctil[step])
                r1row = work.tile([1, DH], BF16, tag="r1row", bufs=2,
                                  name=f"r1row_{step}")
                nc.sync.dma_start(out=r1row, in_=r1_d[:, step, :])

                # ---- layer 1: z1 = gelu(LN(h~ @ W1 + r1~)) ----
                z1ps, z1st, z1t = {}, {}, {}
                z1T = work.tile([P, KD, T_CORE], BF16, tag="z1T", bufs=2,
                                name=f"z1T_{step}")
                for tp in range(TPN):
                    pp = [psp.tile([P, 512], F32, tag="ps",
                                   name=f"ps1_{step}_{tp}_{h}")
                          for h in range(2)]
                    for kc in range(KH):
                        for h in range(2):
                            nc.tensor.matmul(pp[h], hbf[:, kc, ts(tp, P)],
                                             w1s[kc][:, ds(h * 512, 512)],
                                             start=(kc == 0), stop=False)
                    for h in range(2):
                        nc.tensor.matmul(pp[h], ones1,
                                         r1row[:, ds(h * 512, 512)],
                                         start=False, stop=True)
                    st = work.tile([P, 2, 6], F32, tag="st", bufs=4)
                    for h in range(2):
                        nc.vector.bn_stats(out=st[:, h, :], in_=pp[h])
                    z1ps[tp], z1st[tp] = pp, st
                    z1t[tp] = work.tile([P, DH], BF16, tag="z", bufs=3,
                                        name=f"z1_{step}_{tp}")
                    if tp in LN_GROUPS:
                        ln_gelu_group(z1ps, z1st, LN_GROUPS[tp], z1t, z1T,
                                      0 if apply_gb1 else None)

                # ---- layer 2: z2 = gelu(LN(z1 @ W2 (+b2))) ----
                z2ps, z2st, z2t = {}, {}, {}
                z2T = work.tile([P, KD, T_CORE], BF16, tag="z2T", bufs=2,
                                name=f"z2T_{step}")
                for tp in range(TPN):
                    pp = [psp.tile([P, 512], F32, tag="ps",
                                   name=f"ps2_{step}_{tp}_{h}")
                          for h in range(2)]
                    for kc in range(KD):
                        for h in range(2):
                            nc.tensor.matmul(pp[h], z1T[:, kc, ts(tp, P)],
                                             w2s[kc][:, ds(h * 512, 512)],
                                             start=(kc == 0),
                                             stop=(kc == KD - 1 and not use_b2))
                    if use_b2:
                        for h in range(2):
                            nc.tensor.matmul(pp[h], ones1,
                                             b2s[:, ds(h * 512, 512)],
                                             start=False, stop=True)
                    st = work.tile([P, 2, 6], F32, tag="st", bufs=4)
                    for h in range(2):
                        nc.vector.bn_stats(out=st[:, h, :], in_=pp[h])
                    z2ps[tp], z2st[tp] = pp, st
                    z2t[tp] = work.tile([P, DH], BF16, tag="z", bufs=3,
                                        name=f"z2_{step}_{tp}")
                    if tp in LN_GROUPS:
                        ln_gelu_group(z2ps, z2st, LN_GROUPS[tp], z2t, z2T,
                                      2 if apply_gb2 else None)

                # ---- layer 3 (feature-major) + h~ update ----
                # hbf for the next step is produced directly from PSUM per
                # token-half so the next mm1 isn't gated on the f32 master.
                ps3 = [psp.tile([P, 512], F32, tag="ps",
                                name=f"ps3_{step}_{mc}")
                       for mc in range(KH)]
                for hn in range(2):  # token halves so PE starts earlier
                    sl = ds(hn * 256, 256)
                    for mc in range(KH):
                        for kc in range(KD):
                            nc.tensor.matmul(
                                ps3[mc][:, sl], w3s[kc][:, ts(mc, P)],
                                z2T[:, kc, sl],
                                start=(kc == 0),
                                stop=(kc == KD - 1 and not use_b3))
                        if use_b3:
                            nc.tensor.matmul(ps3[mc][:, sl],
                                             b3s[:, ts(mc, P)], onesrow[:, sl],
                                             start=False, stop=True)
                    if step < n_steps - 1:
                        for mc in range(KH):
                            nc.vector.scalar_tensor_tensor(
                                out=hbf[:, mc, sl], in0=ps3[mc][:, sl],
                                scalar=cneg, in1=hT[:, mc, sl],
                                op0=ALU.mult, op1=ALU.add)
                for mc in range(KH):
                    nc.vector.scalar_tensor_tensor(
                        out=hT[:, mc, :], in0=ps3[mc], scalar=cneg,
                        in1=hT[:, mc, :], op0=ALU.mult, op1=ALU.add)

                # prefetch first embt chunks late in diffusion
                if n_steps - 1 - N_PREF <= step < n_steps - 1:
                    load_et(step - (n_steps - 1 - N_PREF))

            # ============ final LN (folded into vocab head) ============
            # token-major copy of h~ for per-token stats
            htok = wpool.tile([P, TPN, KH, P], BF16)
            for kc in range(KH):
                nc.scalar.copy(out=hbf[:, kc, :], in_=hT[:, kc, :])
                nc.sync.dma_start(out=htok[:, :, kc, :], in_=hbf[:, kc, :],
                                  transpose=True)
            mvf = wpool.tile([P, TPN, 2], F32)
            for tp in range(TPN):
                stf = work.tile([P, KH, 6], F32, tag="stf", bufs=4)
                for kc in range(KH):
                    nc.vector.bn_stats(out=stf[:, kc, :],
                                       in_=htok[:, tp, kc, :])
                nc.vector.bn_aggr(out=mvf[:, tp, :], in_=stf)
            rsf, _nbf = rsqrt_chain(mvf, TPN)
            # mu row -> [1, T_CORE] via PE transpose + DMA, then broadcast
            mu4 = wpool.tile([P, TPN], F32)
            nc.vector.tensor_copy(out=mu4, in_=mvf[:, :, 0])
            ptm = psp.tile([P, 512], F32, tag="ps")
            nc.tensor.transpose(ptm[0:TPN, 0:P], mu4, ident)
            mur4 = wpool.tile([P, P], F32, name="mur4")
            nc.vector.tensor_copy(out=mur4[0:TPN, :], in_=ptm[0:TPN, 0:P])
            nc.sync.dma_start(out=mu_d, in_=mur4[0:TPN, :])
            mu_bc = wpool.tile([P, T_CORE], F32)
            nc.sync.dma_start(out=mu_bc, in_=mu_d.to_broadcast([P, T_CORE]))
            for kc in range(KH):
                nc.vector.tensor_tensor(out=hcT[:, kc, :], in0=hT[:, kc, :],
                                        in1=mu_bc, op=ALU.subtract)

            # ================= vocab head =================
            n_vc = (vocab + VC - 1) // VC

            def evac_logits(lo_sl, pl_sl, tp):
                nc.vector.tensor_scalar(out=lo_sl, in0=pl_sl,
                                        scalar1=rsf[:, tp:tp + 1],
                                        scalar2=None, op0=ALU.mult)

            for vc in range(len(ets), min(EMB_BUFS, n_vc)):
                load_et(vc)
            for vc in range(n_vc):
                v0 = vc * VC
                vn = min(VC, vocab - v0)
                et = ets[vc]
                if vc + EMB_BUFS < n_vc:
                    load_et(vc + EMB_BUFS)
                if use_voff:
                    nc.sync.dma_start(
                        out=voff_bc[:, :vn],
                        in_=voff_s[:, v0:v0 + vn].to_broadcast([P, vn]))
                nsl = (vn + 511) // 512
                for tp in range(TPN):
                    # pairs of 512-slices share one lout tile + one DMA out
                    for i0 in range(0, nsl, 2):
                        sls = [i for i in (i0, i0 + 1) if i < nsl]
                        ws = [min(512, vn - i * 512) for i in sls]
                        wtot = sum(ws)
                        pls = [psp.tile([P, 512], F32, tag="ps",
                                        name=f"plv_{vc}_{tp}_{i}")
                               for i in sls]
                        for kc in range(KH):
                            for j, i in enumerate(sls):
                                nc.tensor.matmul(
                                    pls[j][:, :ws[j]], hcT[:, kc, ts(tp, P)],
                                    et[:, kc, ds(i * 512, ws[j])],
                                    start=(kc == 0), stop=(kc == KH - 1))
                        lo = loutp.tile([P, 1024], F32, tag="lo")
                        off = 0
                        for j in range(len(sls)):
                            evac_logits(lo[:, ds(off, ws[j])],
                                        pls[j][:, :ws[j]], tp)
                            off += ws[j]
                        if use_voff:
                            nc.vector.tensor_tensor(
                                out=lo[:, :wtot], in0=lo[:, :wtot],
                                in1=voff_bc[:, ds(i0 * 512, wtot)],
                                op=ALU.add)
                        nc.scalar.dma_start(
                            out=out_d[tp * P:(tp + 1) * P,
                                      v0 + i0 * 512:v0 + i0 * 512 + wtot],
                            in_=lo[:, :wtot])
    nc.compile()
    return nc


def host_prep(x, embed, W1, b1, g1, be1, W2, b2, g2, be2, W3, b3, gn, bn,
              n_steps=N_STEPS):
    """Pure-numpy input prep shared by all cores."""
    x = np.asarray(x).reshape(-1)
    embed = np.asarray(embed, dtype=np.float32)
    W1 = np.asarray(W1, dtype=np.float32)
    b1 = np.asarray(b1, dtype=np.float32)
    t_norm, _, _, A, _ = _step_consts(n_steps)
    h0 = embed[x]                                     # [T_total, HID]
    r1 = ((t_norm[:, None] * W1[HID][None, :] + b1[None, :])
          / A[:, None]).astype(ml_dtypes.bfloat16)[None]
    gnf = np.asarray(gn, dtype=np.float32)
    embt = np.ascontiguousarray(
        (embed * gnf[None, :]).T.astype(ml_dtypes.bfloat16))  # [HID, VOCAB]
    voff = (np.asarray(bn, dtype=np.float32) @ embed.T).astype(np.float32)
    return dict(
        h0=np.ascontiguousarray(h0),
        w1=np.ascontiguousarray(W1[:HID]).astype(ml_dtypes.bfloat16),
        r1=np.ascontiguousarray(r1),
        w2=np.asarray(W2, dtype=np.float32).astype(ml_dtypes.bfloat16),
        w3=np.asarray(W3, dtype=np.float32).astype(ml_dtypes.bfloat16),
        embt=embt,
        b2=np.asarray(b2, dtype=np.float32).astype(
            ml_dtypes.bfloat16).reshape(1, -1),
        b3=np.asarray(b3, dtype=np.float32).astype(
            ml_dtypes.bfloat16).reshape(1, -1),
        voff=voff.reshape(1, -1),
        g1=np.asarray(g1, dtype=np.float32),
        be1=np.asarray(be1, dtype=np.float32),
        g2=np.asarray(g2, dtype=np.float32),
        be2=np.asarray(be2, dtype=np.float32),
    )


_CACHE = {}


def _get_program(key, **kw):
    if key not in _CACHE:
        _CACHE[key] = build_program(**kw)
    return _CACHE[key]


def kernel(x, embed, W1, b1, g1, be1, W2, b2, g2, be2, W3, b3, gn, bn,
           run_kwargs=None):
    pre = host_prep(x, embed, W1, b1, g1, be1, W2, b2, g2, be2, W3, b3,
                    gn, bn)

    apply_gb1 = bool(np.any(pre["g1"] != 1.0) or np.any(pre["be1"] != 0.0))
    apply_gb2 = bool(np.any(pre["g2"] != 1.0) or np.any(pre["be2"] != 0.0))
    use_b2 = bool(np.any(np.asarray(b2)))
    use_b3 = bool(np.any(np.asarray(b3)))
    use_voff = bool(np.any(pre["voff"]))

    key = (apply_gb1, apply_gb2, use_b2, use_b3, use_voff)
    nc = _get_program(key, apply_gb1=apply_gb1, apply_gb2=apply_gb2,
                      use_b2=use_b2, use_b3=use_b3, use_voff=use_voff)

    common = {"w1": pre["w1"], "r1": pre["r1"], "w2": pre["w2"],
              "w3": pre["w3"], "embt": pre["embt"]}
    if use_b2:
        common["b2"] = pre["b2"]
    if use_b3:
        common["b3"] = pre["b3"]
    if use_voff:
        common["voff"] = pre["voff"]
    if apply_gb1 or apply_gb2:
        common["gb"] = np.stack([pre["g1"], pre["be1"], pre["g2"],
                                 pre["be2"]])

    in_maps = []
    for c in range(N_CORES):
        m = dict(common)
        m["h0t"] = np.ascontiguousarray(
            pre["h0"][c * T_CORE:(c + 1) * T_CORE].T)
        in_maps.append(m)

    res = bass_utils.run_bass_kernel_spmd(
        nc, in_maps, core_ids=list(range(N_CORES)), **(run_kwargs or {}))
    out = np.concatenate([res.results[c]["logits"] for c in range(N_CORES)],
                         axis=0)
    kernel.last_results = res
    return out.reshape(B, S, VOCAB)


# revision 25
# speedup vs baseline: 1.3932x; 1.0626x over previous
"""Trainium2 Bass kernel for nn_DiffusionLM (dense_mlp).

Strategy (8 NeuronCores, data-parallel over tokens; 512 tokens/core):
  - Host: embedding gather + pre-transpose h0 -> h0T [HID, T_CORE] f32;
    weights cast to bf16; step-bias table r1[t] and step coefficient
    folded by the cumulative 1/sqrt(alpha) product A_t (LayerNorm is
    scale-invariant, so the per-step `isa` rescale of h is dropped and
    absorbed into r1/A_t and coef/A_t); vocab head (embed*gn).T in bf16.
  - Device diffusion (20 steps), all bf16 matmuls at 1 cycle/row:
      * h~T kept feature-major in SBUF (f32 master + bf16 copy), so the
        first-layer matmul needs no activation transpose.
      * z1/z2 LayerNorm+gelu fused into a single ACT pass per PSUM half:
        Gelu(psum*rstd + (-mu*rstd)) with per-partition scale/bias APs.
      * z1/z2 transposed back to feature-major by DMA XBAR transposes
        (SBUF->SBUF, off the PE).
      * h~ update is one gpsimd scalar_tensor_tensor: h~ += (-c~)*scoreT.
  - Final LN folded into the vocab projection: center h~T by mu (rank-0
    DMA broadcast of the mu row), fold rstd into the PSUM->SBUF evac.
  - Vocab projection streams embt bf16 from HBM (first chunks prefetched
    during diffusion), writes f32 logits.
"""

import numpy as np
import ml_dtypes

import concourse.bass as bass
import concourse.mybir as mybir
import concourse.tile as tile
from concourse import bacc, bass_utils
from concourse.bass import ds, ts
from concourse.masks import make_identity

dt = mybir.dt
F32 = dt.float32
BF16 = dt.bfloat16
I32 = dt.int32
AF = mybir.ActivationFunctionType
ALU = mybir.AluOpType

N_CORES = 8
VOCAB = 32000
HID = 512
DH = 2 * HID  # 1024
N_STEPS = 20
EPS = 1e-5
B, S = 2, 2048
T_TOTAL = B * S              # 4096
T_CORE = T_TOTAL // N_CORES  # 512
P = 128
TPN = T_CORE // P            # 4 token tiles
KH = HID // P                # 4
KD = DH // P                 # 8
RSQRT_MAGIC = 0x5F3759DF
VC = 2048                    # vocab stream chunk
N_PREF = 3                   # chunks prefetched during diffusion
EMB_BUFS = 4                 # embt chunk ring size


def _step_consts(n_steps):
    """Per-step scalars, ordered t = n_steps-1 .. 0, matching reference."""
    betas = np.linspace(0.0001, 0.02, n_steps, dtype=np.float32)
    alphas = (1.0 - betas).astype(np.float32)
    acp = np.cumprod(alphas, dtype=np.float32)
    tsx = np.arange(n_steps - 1, -1, -1)
    t_norm = (tsx.astype(np.float32) / np.float32(n_steps)).astype(np.float32)
    coef = (betas[tsx] / np.sqrt((np.float32(1.0) - acp[tsx]))).astype(np.float32)
    isa = (np.float32(1.0) / np.sqrt(alphas[tsx])).astype(np.float32)
    # A_s = prod_{j<s} isa_j; h = A*h~ and LN() erases the final A.
    A = np.ones(n_steps, dtype=np.float64)
    for s_ in range(1, n_steps):
        A[s_] = A[s_ - 1] * isa[s_ - 1]
    ctil = (coef.astype(np.float64) / A).astype(np.float32)
    return t_norm, coef, isa, A.astype(np.float32), ctil


def build_program(n_steps=N_STEPS, vocab=VOCAB,
                  apply_gb1=False, apply_gb2=False,
                  use_b2=False, use_b3=False, use_voff=False):
    nc = bacc.Bacc("TRN2", target_bir_lowering=False, debug=False,
                   num_devices=N_CORES)

    h0t_d = nc.dram_tensor("h0t", [HID, T_CORE], F32, kind="ExternalInput").ap()
    w1_d = nc.dram_tensor("w1", [HID, DH], BF16, kind="ExternalInput").ap()
    r1_d = nc.dram_tensor("r1", [1, n_steps, DH], BF16,
                          kind="ExternalInput").ap()
    w2_d = nc.dram_tensor("w2", [DH, DH], BF16, kind="ExternalInput").ap()
    w3_d = nc.dram_tensor("w3", [DH, HID], BF16, kind="ExternalInput").ap()
    emb_d = nc.dram_tensor("embt", [HID, vocab], BF16,
                           kind="ExternalInput").ap()
    out_d = nc.dram_tensor("logits", [T_CORE, vocab], F32,
                           kind="ExternalOutput").ap()
    b2_d = b3_d = voff_d = gb_d = None
    if use_b2:
        b2_d = nc.dram_tensor("b2", [1, DH], BF16, kind="ExternalInput").ap()
    if use_b3:
        b3_d = nc.dram_tensor("b3", [1, HID], BF16, kind="ExternalInput").ap()
    if use_voff:
        voff_d = nc.dram_tensor("voff", [1, vocab], F32,
                                kind="ExternalInput").ap()
    if apply_gb1 or apply_gb2:
        gb_d = nc.dram_tensor("gb", [4, DH], F32, kind="ExternalInput").ap()
    mu_d = nc.dram_tensor("mu_scratch", [1, T_CORE], F32,
                          kind="Internal").ap()

    _, _, _, _, ctil = _step_consts(n_steps)

    with tile.TileContext(nc) as tc:
      with (
          tc.tile_pool(name="wpool", bufs=1) as wpool,
          tc.tile_pool(name="work", bufs=3) as work,
          tc.tile_pool(name="emb", bufs=EMB_BUFS) as embp,
          tc.tile_pool(name="lout", bufs=4) as loutp,
          tc.tile_pool(name="ps", bufs=8, space="PSUM") as psp,
      ):
            # ---- resident constants / weights ----
            ones1 = wpool.tile([1, P], BF16)
            nc.vector.memset(ones1, 1.0)
            ident = wpool.tile([P, P], F32)
            make_identity(nc, ident)

            # h0/w1 first on SP queue (needed by step 0); w2/w3 on ACT queue
            hT = wpool.tile([P, KH, T_CORE], F32)
            for kc in range(KH):
                nc.sync.dma_start(out=hT[:, kc, :],
                                  in_=h0t_d[kc * P:(kc + 1) * P, :])
            w1s = []
            for kc in range(KH):
                w = wpool.tile([P, DH], BF16, tag=f"w1_{kc}")
                nc.sync.dma_start(out=w, in_=w1_d[kc * P:(kc + 1) * P, :])
                w1s.append(w)
            w2s = []
            for kc in range(KD):
                w = wpool.tile([P, DH], BF16, tag=f"w2_{kc}")
                nc.scalar.dma_start(out=w, in_=w2_d[kc * P:(kc + 1) * P, :])
                w2s.append(w)
            w3s = []
            for kc in range(KD):
                w = wpool.tile([P, HID], BF16, tag=f"w3_{kc}")
                nc.scalar.dma_start(out=w, in_=w3_d[kc * P:(kc + 1) * P, :])
                w3s.append(w)

            b2s = b3s = voff_s = onesrow = gbs = None
            if use_b2:
                b2s = wpool.tile([1, DH], BF16)
                nc.sync.dma_start(out=b2s, in_=b2_d)
            if use_b3:
                b3s = wpool.tile([1, HID], BF16)
                nc.sync.dma_start(out=b3s, in_=b3_d)
                onesrow = wpool.tile([1, T_CORE], BF16)
                nc.vector.memset(onesrow, 1.0)
            if use_voff:
                voff_s = wpool.tile([1, vocab], F32)
                nc.sync.dma_start(out=voff_s, in_=voff_d)
                voff_bc = wpool.tile([P, VC], F32)
            if gb_d is not None:
                gbs = wpool.tile([P, 4, DH], F32)
                nc.sync.dma_start(out=gbs, in_=gb_d.to_broadcast([P, 4, DH]))

            # persistent h~T (feature-major): f32 master + bf16 matmul copy
            # (initial casts on DVE: the ACT queue is busy loading w2/w3)
            hbf = wpool.tile([P, KH, T_CORE], BF16)
            for kc in range(KH):
                nc.vector.tensor_copy(out=hbf[:, kc, :], in_=hT[:, kc, :])
            hcT = wpool.tile([P, KH, T_CORE], BF16)
            magict = wpool.tile([P, TPN], I32)
            nc.vector.memset(magict, RSQRT_MAGIC)

            def rsqrt_chain(mvp, n):
                """DVE chain on [P,n]: returns (rstd, negbias) tiles.

                mvp is [P,n,2] f32 (mean, var) from bn_aggr."""
                u = work.tile([P, n], F32, tag="u", bufs=4)
                yv = work.tile([P, n], F32, tag="yv", bufs=4)
                t2 = work.tile([P, n], F32, tag="t2", bufs=4)
                nb = work.tile([P, n], F32, tag="nb", bufs=4)
                nc.vector.tensor_scalar(out=u, in0=mvp[:, :, 1], scalar1=EPS,
                                        scalar2=None, op0=ALU.add)
                nc.vector.tensor_scalar(out=t2.bitcast(I32),
                                        in0=u.bitcast(I32), scalar1=1,
                                        scalar2=None,
                                        op0=ALU.logical_shift_right)
                nc.vector.tensor_tensor(out=yv.bitcast(I32),
                                        in0=magict[:, :n],
                                        in1=t2.bitcast(I32), op=ALU.subtract)
                # Newton 1: rstd = est * (1.5 - 0.5*u*est^2)
                nc.vector.tensor_tensor(out=t2, in0=yv, in1=yv, op=ALU.mult)
                nc.vector.tensor_tensor(out=t2, in0=t2, in1=u, op=ALU.mult)
                nc.vector.tensor_scalar(out=t2, in0=t2, scalar1=-0.5,
                                        scalar2=1.5, op0=ALU.mult, op1=ALU.add)
                nc.vector.tensor_tensor(out=yv, in0=yv, in1=t2, op=ALU.mult)
                # nb = -mean * rstd
                nc.vector.scalar_tensor_tensor(out=nb, in0=mvp[:, :, 0],
                                               scalar=-1.0, in1=yv,
                                               op0=ALU.mult, op1=ALU.mult)
                return yv, nb

            # LN-chain emission groups: tile 0 alone (unblocks ACT/PE
            # early), then (1,2), then 3.
            LN_GROUPS = {0: (0,), 2: (1, 2), 3: (3,)}

            def ln_gelu_group(pps, sts, group, zts, zT, gb_idx):
                """Stats-chain + fused LN/gelu + DMA transpose per group."""
                ng = len(group)
                mvp = work.tile([P, ng, 2], F32, tag=f"mv{ng}", bufs=4)
                for i, t in enumerate(group):
                    nc.vector.bn_aggr(out=mvp[:, i, :], in_=sts[t])
                rstd, nb = rsqrt_chain(mvp, ng)
                for i, t in enumerate(group):
                    if gb_idx is None:
                        for h in range(2):
                            nc.scalar.activation(
                                out=zts[t][:, ds(h * 512, 512)],
                                in_=pps[t][h], func=AF.Gelu,
                                scale=rstd[:, i:i + 1], bias=nb[:, i:i + 1])
                            # alternate the issuing HWDGE queue (SP/ACT)
                            eng = nc.sync if h == 0 else nc.scalar
                            eng.dma_start(
                                out=zT[:, ds(h * KH, KH), ts(t, P)],
                                in_=zts[t][:, ds(h * 512, 512)],
                                transpose=True)
                    else:
                        # general path: g/be per-feature after LN
                        zf = work.tile([P, DH], F32, tag="zf", bufs=2)
                        for h in range(2):
                            nc.vector.tensor_scalar(
                                out=zf[:, ds(h * 512, 512)], in0=pps[t][h],
                                scalar1=mvp[:, i, 0:1],
                                scalar2=rstd[:, i:i + 1],
                                op0=ALU.subtract, op1=ALU.mult)
                        g_t = gbs[:, gb_idx, :]
                        be_t = gbs[:, gb_idx + 1, :]
                        nc.vector.tensor_tensor(out=zf, in0=zf, in1=g_t,
                                                op=ALU.mult)
                        nc.vector.tensor_tensor(out=zf, in0=zf, in1=be_t,
                                                op=ALU.add)
                        nc.scalar.activation(out=zts[t], in_=zf, func=AF.Gelu)
                        nc.sync.dma_start(out=zT[:, :, ts(t, P)], in_=zts[t],
                                          transpose=True)

            # ================= diffusion =================
            ets = []

            def load_et(vc):
                v0e = vc * VC
                vne = min(VC, vocab - v0e)
                et = embp.tile([P, KH, VC], BF16, tag="et",
                               name=f"et_{vc}")
                for kc in range(KH):
                    nc.sync.dma_start(
                        out=et[:, kc, :vne],
                        in_=emb_d[kc * P:(kc + 1) * P, v0e:v0e + vne])
                ets.append(et)
            for step in range(n_steps):
                cneg = -float(ctil[step])
                r1row = work.tile([1, DH], BF16, tag="r1row", bufs=2,
                                  name=f"r1row_{step}")
                nc.sync.dma_start(out=r1row, in_=r1_d[:, step, :])

                # ---- layer 1: z1 = gelu(LN(h~ @ W1 + r1~)) ----
                z1ps, z1st, z1t = {}, {}, {}
                z1T = work.tile([P, KD, T_CORE], BF16, tag="z1T", bufs=2,
                                name=f"z1T_{step}")
                for tp in range(TPN):
                    pp = [psp.tile([P, 512], F32, tag="ps",
                                   name=f"ps1_{step}_{tp}_{h}")
                          for h in range(2)]
                    for kc in range(KH):
                        for h in range(2):
                            nc.tensor.matmul(pp[h], hbf[:, kc, ts(tp, P)],
                                             w1s[kc][:, ds(h * 512, 512)],
                                             start=(kc == 0), stop=False)
                    for h in range(2):
                        nc.tensor.matmul(pp[h], ones1,
                                         r1row[:, ds(h * 512, 512)],
                                         start=False, stop=True)
                    st = work.tile([P, 2, 6], F32, tag="st", bufs=4)
                    for h in range(2):
                        nc.vector.bn_stats(out=st[:, h, :], in_=pp[h])
                    z1ps[tp], z1st[tp] = pp, st
                    z1t[tp] = work.tile([P, DH], BF16, tag="z", bufs=3,
                                        name=f"z1_{step}_{tp}")
                    if tp in LN_GROUPS:
                        ln_gelu_group(z1ps, z1st, LN_GROUPS[tp], z1t, z1T,
                                      0 if apply_gb1 else None)

                # ---- layer 2: z2 = gelu(LN(z1 @ W2 (+b2))) ----
                z2ps, z2st, z2t = {}, {}, {}
                z2T = work.tile([P, KD, T_CORE], BF16, tag="z2T", bufs=2,
                                name=f"z2T_{step}")
                for tp in range(TPN):
                    pp = [psp.tile([P, 512], F32, tag="ps",
                                   name=f"ps2_{step}_{tp}_{h}")
                          for h in range(2)]
                    for kc in range(KD):
                        for h in range(2):
                            nc.tensor.matmul(pp[h], z1T[:, kc, ts(tp, P)],
                                             w2s[kc][:, ds(h * 512, 512)],
                                             start=(kc == 0),
                                             stop=(kc == KD - 1 and not use_b2))
                    if use_b2:
                        for h in range(2):
                            nc.tensor.matmul(pp[h], ones1,
                                             b2s[:, ds(h * 512, 512)],
                                             start=False, stop=True)
                    st = work.tile([P, 2, 6], F32, tag="st", bufs=4)
                    for h in range(2):
                        nc.vector.bn_stats(out=st[:, h, :], in_=pp[h])
                    z2ps[tp], z2st[tp] = pp, st
                    z2t[tp] = work.tile([P, DH], BF16, tag="z", bufs=3,
                                        name=f"z2_{step}_{tp}")
                    if tp in LN_GROUPS:
                        ln_gelu_group(z2ps, z2st, LN_GROUPS[tp], z2t, z2T,
                                      2 if apply_gb2 else None)

                # ---- layer 3 (feature-major) + h~ update ----
                # hbf for the next step is produced directly from PSUM per
                # token-half so the next mm1 isn't gated on the f32 master.
                ps3 = [psp.tile([P, 512], F32, tag="ps",
                                name=f"ps3_{step}_{mc}")
                       for mc in range(KH)]
                for hn in range(2):  # token halves so PE starts earlier
                    sl = ds(hn * 256, 256)
                    for mc in range(KH):
                        for kc in range(KD):
                            nc.tensor.matmul(
                                ps3[mc][:, sl], w3s[kc][:, ts(mc, P)],
                                z2T[:, kc, sl],
                                start=(kc == 0),
                                stop=(kc == KD - 1 and not use_b3))
                        if use_b3:
                            nc.tensor.matmul(ps3[mc][:, sl],
                                             b3s[:, ts(mc, P)], onesrow[:, sl],
                                             start=False, stop=True)
                    if step < n_steps - 1:
                        for mc in range(KH):
                            nc.vector.scalar_tensor_tensor(
                                out=hbf[:, mc, sl], in0=ps3[mc][:, sl],
                                scalar=cneg, in1=hT[:, mc, sl],
                                op0=ALU.mult, op1=ALU.add)
                # f32 master update off the DVE: stage score via ACT copy,
                # then gpsimd does hT += cneg*score (SBUF-only operands).
                for mc in range(KH):
                    sc = work.tile([P, 512], F32, tag="sc", bufs=2,
                                   name=f"sc_{step}_{mc}")
                    nc.scalar.mul(sc, ps3[mc], cneg)
                    nc.gpsimd.tensor_tensor(out=hT[:, mc, :],
                                            in0=hT[:, mc, :], in1=sc,
                                            op=ALU.add)

                # prefetch first embt chunks late in diffusion
                if n_steps - 1 - N_PREF <= step < n_steps - 1:
                    load_et(step - (n_steps - 1 - N_PREF))

            # ============ final LN (folded into vocab head) ============
            # token-major copy of h~ for per-token stats
            htok = wpool.tile([P, TPN, KH, P], BF16)
            for kc in range(KH):
                nc.scalar.copy(out=hbf[:, kc, :], in_=hT[:, kc, :])
                nc.sync.dma_start(out=htok[:, :, kc, :], in_=hbf[:, kc, :],
                                  transpose=True)
            mvf = wpool.tile([P, TPN, 2], F32)
            for tp in range(TPN):
                stf = work.tile([P, KH, 6], F32, tag="stf", bufs=4)
                for kc in range(KH):
                    nc.vector.bn_stats(out=stf[:, kc, :],
                                       in_=htok[:, tp, kc, :])
                nc.vector.bn_aggr(out=mvf[:, tp, :], in_=stf)
            rsf, _nbf = rsqrt_chain(mvf, TPN)
            # mu row -> [1, T_CORE] via PE transpose + DMA, then broadcast
            mu4 = wpool.tile([P, TPN], F32)
            nc.vector.tensor_copy(out=mu4, in_=mvf[:, :, 0])
            ptm = psp.tile([P, 512], F32, tag="ps")
            nc.tensor.transpose(ptm[0:TPN, 0:P], mu4, ident)
            mur4 = wpool.tile([P, P], F32, name="mur4")
            nc.vector.tensor_copy(out=mur4[0:TPN, :], in_=ptm[0:TPN, 0:P])
            nc.sync.dma_start(out=mu_d, in_=mur4[0:TPN, :])
            mu_bc = wpool.tile([P, T_CORE], F32)
            nc.sync.dma_start(out=mu_bc, in_=mu_d.to_broadcast([P, T_CORE]))
            for kc in range(KH):
                nc.vector.tensor_tensor(out=hcT[:, kc, :], in0=hT[:, kc, :],
                                        in1=mu_bc, op=ALU.subtract)

            # ================= vocab head =================
            n_vc = (vocab + VC - 1) // VC

            def evac_logits(lo_sl, pl_sl, tp):
                nc.vector.tensor_scalar(out=lo_sl, in0=pl_sl,
                                        scalar1=rsf[:, tp:tp + 1],
                                        scalar2=None, op0=ALU.mult)

            for vc in range(len(ets), min(EMB_BUFS, n_vc)):
                load_et(vc)
            for vc in range(n_vc):
                v0 = vc * VC
                vn = min(VC, vocab - v0)
                et = ets[vc]
                if vc + EMB_BUFS < n_vc:
                    load_et(vc + EMB_BUFS)
                if use_voff:
                    nc.sync.dma_start(
                        out=voff_bc[:, :vn],
                        in_=voff_s[:, v0:v0 + vn].to_broadcast([P, vn]))
                nsl = (vn + 511) // 512
                for tp in range(TPN):
                    # pairs of 512-slices share one lout tile + one DMA out
                    for i0 in range(0, nsl, 2):
                        sls = [i for i in (i0, i0 + 1) if i < nsl]
                        ws = [min(512, vn - i * 512) for i in sls]
                        wtot = sum(ws)
                        pls = [psp.tile([P, 512], F32, tag="ps",
                                        name=f"plv_{vc}_{tp}_{i}")
                               for i in sls]
                        for kc in range(KH):
                            for j, i in enumerate(sls):
                                nc.tensor.matmul(
                                    pls[j][:, :ws[j]], hcT[:, kc, ts(tp, P)],
                                    et[:, kc, ds(i * 512, ws[j])],
                                    start=(kc == 0), stop=(kc == KH - 1))
                        lo = loutp.tile([P, 1024], F32, tag="lo")
                        off = 0
                        for j in range(len(sls)):
                            evac_logits(lo[:, ds(off, ws[j])],
                                        pls[j][:, :ws[j]], tp)
                            off += ws[j]
                        if use_voff:
                            nc.vector.tensor_tensor(
                                out=lo[:, :wtot], in0=lo[:, :wtot],
                                in1=voff_bc[:, ds(i0 * 512, wtot)],
                                op=ALU.add)
                        nc.scalar.dma_start(
                            out=out_d[tp * P:(tp + 1) * P,
                                      v0 + i0 * 512:v0 + i0 * 512 + wtot],
                            in_=lo[:, :wtot])
    nc.compile()
    return nc


def host_prep(x, embed, W1, b1, g1, be1, W2, b2, g2, be2, W3, b3, gn, bn,
              n_steps=N_STEPS):
    """Pure-numpy input prep shared by all cores."""
    x = np.asarray(x).reshape(-1)
    embed = np.asarray(embed, dtype=np.float32)
    W1 = np.asarray(W1, dtype=np.float32)
    b1 = np.asarray(b1, dtype=np.float32)
    t_norm, _, _, A, _ = _step_consts(n_steps)
    h0 = embed[x]                                     # [T_total, HID]
    r1 = ((t_norm[:, None] * W1[HID][None, :] + b1[None, :])
          / A[:, None]).astype(ml_dtypes.bfloat16)[None]
    gnf = np.asarray(gn, dtype=np.float32)
    embt = np.ascontiguousarray(
        (embed * gnf[None, :]).T.astype(ml_dtypes.bfloat16))  # [HID, VOCAB]
    voff = (np.asarray(bn, dtype=np.float32) @ embed.T).astype(np.float32)
    return dict(
        h0=np.ascontiguousarray(h0),
        w1=np.ascontiguousarray(W1[:HID]).astype(ml_dtypes.bfloat16),
        r1=np.ascontiguousarray(r1),
        w2=np.asarray(W2, dtype=np.float32).astype(ml_dtypes.bfloat16),
        w3=np.asarray(W3, dtype=np.float32).astype(ml_dtypes.bfloat16),
        embt=embt,
        b2=np.asarray(b2, dtype=np.float32).astype(
            ml_dtypes.bfloat16).reshape(1, -1),
        b3=np.asarray(b3, dtype=np.float32).astype(
            ml_dtypes.bfloat16).reshape(1, -1),
        voff=voff.reshape(1, -1),
        g1=np.asarray(g1, dtype=np.float32),
        be1=np.asarray(be1, dtype=np.float32),
        g2=np.asarray(g2, dtype=np.float32),
        be2=np.asarray(be2, dtype=np.float32),
    )


_CACHE = {}


def _get_program(key, **kw):
    if key not in _CACHE:
        _CACHE[key] = build_program(**kw)
    return _CACHE[key]


def kernel(x, embed, W1, b1, g1, be1, W2, b2, g2, be2, W3, b3, gn, bn,
           run_kwargs=None):
    pre = host_prep(x, embed, W1, b1, g1, be1, W2, b2, g2, be2, W3, b3,
                    gn, bn)

    apply_gb1 = bool(np.any(pre["g1"] != 1.0) or np.any(pre["be1"] != 0.0))
    apply_gb2 = bool(np.any(pre["g2"] != 1.0) or np.any(pre["be2"] != 0.0))
    use_b2 = bool(np.any(np.asarray(b2)))
    use_b3 = bool(np.any(np.asarray(b3)))
    use_voff = bool(np.any(pre["voff"]))

    key = (apply_gb1, apply_gb2, use_b2, use_b3, use_voff)
    nc = _get_program(key, apply_gb1=apply_gb1, apply_gb2=apply_gb2,
                      use_b2=use_b2, use_b3=use_b3, use_voff=use_voff)

    common = {"w1": pre["w1"], "r1": pre["r1"], "w2": pre["w2"],
              "w3": pre["w3"], "embt": pre["embt"]}
    if use_b2:
        common["b2"] = pre["b2"]
    if use_b3:
        common["b3"] = pre["b3"]
    if use_voff:
        common["voff"] = pre["voff"]
    if apply_gb1 or apply_gb2:
        common["gb"] = np.stack([pre["g1"], pre["be1"], pre["g2"],
                                 pre["be2"]])

    in_maps = []
    for c in range(N_CORES):
        m = dict(common)
        m["h0t"] = np.ascontiguousarray(
            pre["h0"][c * T_CORE:(c + 1) * T_CORE].T)
        in_maps.append(m)

    res = bass_utils.run_bass_kernel_spmd(
        nc, in_maps, core_ids=list(range(N_CORES)), **(run_kwargs or {}))
    out = np.concatenate([res.results[c]["logits"] for c in range(N_CORES)],
                         axis=0)
    kernel.last_results = res
    return out.reshape(B, S, VOCAB)
